# revision 45
# baseline (speedup 1.0000x reference)
"""Trainium2 Bass kernel for nn_AttentionBlock (masked GroupNorm + jagged full attention).

Contract: kernel(**inputs) takes FULL unsharded inputs (as in reference.setup_inputs())
and returns the FULL [8, 1024, 512] fp32 output. Data-parallel over batch:
sample b -> NeuronCore b (8 cores).

Design: exp-rate-limited software pipeline (~118us/core, vs 213us baseline).
The softmax exp (8.4M elements/core) on the Activation engine is the hard
floor (64 calls x ~1.1us); everything else hides under it:
  - All dense matmuls (QKV, V, AV, proj) in fp8e4m3 with DoubleRow perf mode
    (2 contraction tiles packed per pass). Weights scaled x64 on host for fp8
    range; attention outputs x32; rescaled on PSUM->SBUF copies. Scores stay
    bf16 with two heads packed into PE row-halves per 128-token k-tile.
  - AV lhsT uses a sliding window over [v_h0 |mask| v_h1 | ...]: each DoubleRow
    AV matmul (must write all 128 dst partitions) yields 64 rows of AV plus 64
    replicated rows of the masked softmax denominator for free; normalize is a
    3-op DVE chain with no partition broadcast.
  - One exp slot per iteration runs on DVE via the Schraudolph bit trick:
    int8(score*log2e + 56) bit-cast to fp8e4m3 approximates exp(score/8)
    (softmax cancels the constant rounding bias; the mantissa-linear noise is
    in-family with fp8 quantization). This balances ScalarE (~70us) and DVE
    (~70us) instead of ScalarE being the lone 78us ceiling.
  - PSUM: 2x2 banks score/exp ping-pong, 2 AV accumulators, 2 general banks.
  - GroupNorm stats split DVE(bn_stats)/ScalarE(Copy+Square accumulate);
    normalize split DVE/ScalarE-Identity. One act-table swap total.
  - Per k-tile slot: scores+exp emitted first, then one deferred heavy PE op
    (prev-iteration AV tail, this iteration's AV, or a QKV/proj filler), so
    the in-order PE never queues scores behind bulk work.
  - Batched DMAs; bf16 in/out (output upcast on host).
"""

import numpy as np
import ml_dtypes
from contextlib import ExitStack

B, L, C, G, H = 8, 1024, 512, 32, 8
DH = C // H          # 64
CPG = C // G         # 16
EPS = 1e-5
NT = L // 128        # 8 token tiles
CT = C // 128        # 4 channel tiles
QC = L // 512        # 2 query chunks
WS = 64.0            # fp8 weight scale
AS = 32.0            # fp8 attention-output scale
LOG2E = 1.4426950408889634
DVE_EXP_SLOTS = (4,)  # per-iteration kt slots whose exp runs on DVE

BF16 = ml_dtypes.bfloat16
FP8 = ml_dtypes.float8_e4m3

_CACHE = {}


def _build():
    import concourse.tile as tile
    from concourse import bacc, mybir

    f32 = mybir.dt.float32
    bf16 = mybir.dt.bfloat16
    fp8 = mybir.dt.float8e4
    i8 = mybir.dt.int8
    Alu = mybir.AluOpType
    Act = mybir.ActivationFunctionType
    DR = mybir.MatmulPerfMode.DoubleRow

    nc = bacc.Bacc("TRN2", target_bir_lowering=False)

    # ---- per-core DRAM inputs (host-prepped; all shaped to match SBUF) ----
    xmT_d = nc.dram_tensor("xmT", [128, CT, L], bf16, kind="ExternalInput")
    cA_d = nc.dram_tensor("cA", [128, 28 + CT * G], f32, kind="ExternalInput")
    sT_d = nc.dram_tensor("sT", [G, C + 1], f32, kind="ExternalInput")
    qmv_d = nc.dram_tensor("qmv", [1, L], f32, kind="ExternalInput")
    w8A_d = nc.dram_tensor("w8A", [128, 6144], fp8, kind="ExternalInput")
    w8B_d = nc.dram_tensor("w8B", [128, 2048], fp8, kind="ExternalInput")
    xm_d = nc.dram_tensor("xmr", [128, NT, C], bf16, kind="ExternalInput")
    out_d = nc.dram_tensor("out", [128, NT, C], bf16, kind="ExternalOutput")

    with tile.TileContext(nc) as tc, ExitStack() as ctx:
        pc = ctx.enter_context(tc.tile_pool(name="consts", bufs=1))
        pb = ctx.enter_context(tc.tile_pool(name="big", bufs=1))
        ps = ctx.enter_context(tc.tile_pool(name="psum", bufs=1, space="PSUM"))

        # ---- input loads (xmT first: it gates GroupNorm; chunk order matches
        # the DVE/ACT stats split so both engines start early) ----
        xmT = pb.tile([128, CT, L], bf16, tag="xmT", name="xmT")
        cA = pc.tile([128, 28 + CT * G], f32, tag="cA", name="cA")
        sT = pc.tile([G, C + 1], f32, tag="sT", name="sT")
        # xmT tiles 2,3 ride the gpsimd DMA queue in parallel with 0,1 on sync,
        # so the ScalarE stats (tiles 2,3) start ~4us earlier
        nc.gpsimd.dma_start(xmT[:, 2, :], xmT_d[:, 2, :])
        nc.gpsimd.dma_start(xmT[:, 3, :], xmT_d[:, 3, :])
        nc.sync.dma_start(xmT[:, 0, :], xmT_d[:, 0, :])
        nc.sync.dma_start(cA[:], cA_d[:])
        nc.sync.dma_start(sT[:], sT_d[:])
        nc.sync.dma_start(xmT[:, 1, :], xmT_d[:, 1, :])
        qmv = pc.tile([1, L], f32, tag="qmv", name="qmv")
        nc.sync.dma_start(qmv[:], qmv_d[:])
        w8A = pc.tile([128, 6144], fp8, tag="w8A", name="w8A")
        nc.sync.dma_start(w8A[:], w8A_d[:])
        w8B = pc.tile([128, 2048], fp8, tag="w8B", name="w8B")
        nc.sync.dma_start(w8B[:], w8B_d[:])
        xm = pb.tile([128, NT, C], bf16, tag="xm", name="xm")
        nc.sync.dma_start(xm[:], xm_d[:])

        bqk = cA[:, 0:8]              # per-cout-tile qk bias (beta folded)
        gam = cA[:, 8:12]             # gamma per channel-tile
        vmsk = cA[:, 12:20]           # token-validity/WS per k-tile (v scale)
        vraw = cA[:, 20:28]           # raw token-validity per k-tile (denom mask)
        def sel(t):                   # [128, G] channel->group one-hot
            return cA[:, 28 + G * t:28 + G * (t + 1)]
        selT = sT[:, 0:C]             # [G, C] group->channel one-hot
        icnt = sT[:, C:C + 1]         # [G, 1] 1/(len*cpg)

        def wqk(pair, ot):            # lhsT [128, 2, 128] fp8 (q/k couts)
            a = w8A[:, 2048 * pair:2048 * (pair + 1)]
            return a.rearrange("p (t m) -> p t m", t=2)[:, :, 128 * ot:128 * (ot + 1)]

        def wv(pair):                 # rhs [128, 2, 512] fp8 (v couts)
            a = w8A[:, 4096 + 1024 * pair:4096 + 1024 * (pair + 1)]
            return a.rearrange("p (t m) -> p t m", t=2)

        def wp(u):                    # rhs [128, 2, 512] fp8 (proj couts)
            a = w8B[:, 1024 * u:1024 * (u + 1)]
            return a.rearrange("p (t m) -> p t m", t=2)

        # ---- Phase 1: GroupNorm (stats over valid tokens; zeros from host masking)
        # Stats split across engines: tiles 0,1 via DVE bn_stats; tiles 2,3 via
        # ScalarE Copy/Square with free-dim accumulate. All ScalarE functions
        # used in this kernel (Copy/Square/Identity/Exp) live in one act table,
        # so there are no mid-kernel table swaps.
        smm = [pb.tile([128, 2], f32, tag=f"smm{t}", name=f"smm{t}") for t in range(CT)]
        scr = pb.tile([128, L], f32, tag="scr", name="scr")
        # dummy Sqrt first: steers the initial act-table load to the set that
        # also holds Copy/Square/Identity, so only one swap (to Exp) remains
        dum = pb.tile([1, 1], f32, tag="dum", name="dum")
        nc.vector.memset(dum[:], 1.0)
        nc.scalar.activation(dum[:], dum[:], Act.Sqrt, bias=0.0, scale=1.0)
        for t in range(2):
            bns = pb.tile([128, 2, 6], f32, tag="bns", name="bns")
            nc.vector.bn_stats(bns[:, 0, :], xmT[:, t, 0:512])
            nc.vector.bn_stats(bns[:, 1, :], xmT[:, t, 512:1024])
            mv = pb.tile([128, 2], f32, tag="mv", name="mv")
            nc.vector.bn_aggr(mv[:], bns[:])
            sq = pb.tile([128, 1], f32, tag="sq", name="sq")
            nc.vector.tensor_mul(sq[:], mv[:, 0:1], mv[:, 0:1])
            # smm = [sum(x), sum(x^2)] recovered from mean/var over all 1024 (incl. zeros)
            nc.vector.tensor_scalar(smm[t][:, 0:1], mv[:, 0:1], float(L), None, Alu.mult)
            nc.vector.tensor_scalar(smm[t][:, 1:2], mv[:, 1:2], sq[:, 0:1], float(L), Alu.add, Alu.mult)
        for t in range(2, CT):
            nc.scalar.activation(scr[:], xmT[:, t, :], Act.Copy, bias=0.0,
                                 accum_out=smm[t][:, 0:1])
            nc.scalar.activation(scr[:], xmT[:, t, :], Act.Square,
                                 accum_out=smm[t][:, 1:2])
        ps_g = ps.tile([G, 2], f32, tag="mm", name="psg", bufs=2)
        for t in range(CT):
            nc.tensor.matmul(ps_g[:], sel(t), smm[t][:], start=(t == 0), stop=(t == CT - 1))
        grp = pb.tile([G, 2], f32, tag="grp", name="grp")      # [mean_g, rstd_g]
        me2 = pb.tile([G, 2], f32, tag="me2", name="me2")      # [mean_g, E[x^2]_g]
        nc.vector.tensor_scalar(me2[:], ps_g[:], icnt, None, Alu.mult)
        nc.vector.tensor_copy(grp[:, 0:1], me2[:, 0:1])
        ex2 = me2[:, 1:2]
        mm2 = pb.tile([G, 1], f32, tag="mm2", name="mm2")
        nc.vector.tensor_mul(mm2[:], me2[:, 0:1], me2[:, 0:1])
        varep = pb.tile([G, 1], f32, tag="veps", name="veps")
        nc.vector.scalar_tensor_tensor(varep[:], ex2[:], EPS, mm2[:], Alu.add, Alu.subtract)
        sd = pb.tile([G, 1], f32, tag="sd", name="sd")
        nc.scalar.activation(sd[:], varep[:], Act.Sqrt, bias=0.0, scale=1.0)
        nc.vector.reciprocal(grp[:, 1:2], sd[:])

        # xn in fp8, channel-pair-packed for DoubleRow: xn8[pair][p, t, l].
        # All four tiles normalized on DVE: the two ScalarE-Identity variants
        # queued behind ScalarE's stats until ~16.7us and gated the first exp;
        # on DVE they finish ~13us and the act-table swap to Exp happens right
        # after Sqrt(sd), off the critical path.
        # Normalize in column halves: the upfront qk units (qc=0) only read
        # xn8[:, :, 0:512], so emitting all four half-0 tiles first gets the
        # first scores (and exp stream) going ~2.5us earlier; half-1 follows.
        xn8 = [pb.tile([128, 2, L], fp8, tag=f"xn8{u}", name=f"xn8{u}") for u in range(2)]
        rg = pb.tile([128, CT], f32, tag="rg", name="rg")
        mnS = pb.tile([128, CT], f32, tag="mnS", name="mnS")
        for t in range(CT):
            ps_b = ps.tile([128, 2], f32, tag="mm", name="psb", bufs=2)
            nc.tensor.matmul(ps_b[:], selT[:, 128 * t:128 * (t + 1)], grp[:], start=True, stop=True)
            nc.vector.tensor_mul(rg[:, t:t + 1], ps_b[:, 1:2], gam[:, t:t + 1])
            nc.vector.tensor_copy(mnS[:, t:t + 1], ps_b[:, 0:1])
            nc.vector.tensor_scalar(xn8[t // 2][:, t % 2, 0:512], xmT[:, t, 0:512],
                                    ps_b[:, 0:1], rg[:, t:t + 1], Alu.subtract, Alu.mult)

        def xn8_half1(t):
            nc.vector.tensor_scalar(xn8[t // 2][:, t % 2, 512:1024], xmT[:, t, 512:1024],
                                    mnS[:, t:t + 1], rg[:, t:t + 1], Alu.subtract, Alu.mult)

        # ---- Phase 2: QKV in fp8 DoubleRow ----
        # v8[u] per k-tile-slot t: [v_h0 | mask | v_h1 | v_h2 | mask | v_h3 | ...]
        # (192 cols per head pair). The AV matmul for head 2p uses the window
        # [192p, 192p+128) = [v | mask], head 2p+1 uses [192p+64, 192p+192) =
        # [mask | v]: a DoubleRow matmul must write all 128 dst partitions, so
        # the mask half computes the softmax denominator in the spare rows.
        qkT = [pb.tile([128, L], bf16, tag=f"qkT{ot}", name=f"qkT{ot}") for ot in range(8)]
        v8 = [pb.tile([128, 2, 768], fp8, tag=f"v8{u}", name=f"v8{u}") for u in range(4)]
        psum_cycle = ["sA", "sB", "mm"]
        pcount = [0]

        def qk_unit(ot, qc, tg=None):
            if tg is None:
                tg = psum_cycle[pcount[0] % 3]; pcount[0] += 1
            pq = ps.tile([128, 512], f32, tag=tg, name="pq", bufs=2 if tg == "mm" else 1)
            for pr in range(2):
                nc.tensor.matmul(pq[:], wqk(pr, ot), xn8[pr][:, :, 512 * qc:512 * (qc + 1)],
                                 start=(pr == 0), stop=(pr == 1), perf_mode=DR)
            # qkT = pq/WS + bias (bias has beta folded in)
            nc.vector.tensor_scalar(qkT[ot][:, 512 * qc:512 * (qc + 1)], pq[:],
                                    1.0 / WS, bqk[:, ot:ot + 1], Alu.mult, Alu.add)

        def v_unit(kt, tg=None):
            if tg is None:
                tg = psum_cycle[pcount[0] % 3]; pcount[0] += 1
            pv = ps.tile([128, 512], f32, tag=tg, name="pv", bufs=2 if tg == "mm" else 1)
            for pr in range(2):
                nc.tensor.matmul(pv[:], xn8[pr][:, :, 128 * kt:128 * (kt + 1)], wv(pr),
                                 start=(pr == 0), stop=(pr == 1), perf_mode=DR)
            # v8 = pv * mf/WS  (padded-token rows zeroed); one strided DVE copy
            # into the [v_h0 | _ | v_h1] block structure (ACT stays exp-only)
            vt3 = v8[kt // 2][:, kt % 2, :].rearrange("p (a b d) -> p a b d", a=4, b=3)
            pvh = pv[:].rearrange("p (a j d) -> p a j d", a=4, j=2)
            nc.vector.tensor_scalar(vt3[:, :, 0:3:2, :], pvh[:], vmsk[:, kt:kt + 1],
                                    None, Alu.mult)
            # denominator mask columns (exact 1.0/0.0 in fp8)
            nc.gpsimd.tensor_copy(vt3[:, :, 1, :],
                                  vraw[:, kt:kt + 1].to_broadcast((128, 4, 64)))

        # Upfront: only what the first iterations' scores need. Everything
        # else (v units, remaining qk) drains through iteration-0 filler slots
        # so the first score matmuls aren't queued behind it on the in-order PE.
        qk_unit(4, 0, tg="sA")   # the s01(kt0) sA WAR is exactly this copy
        qk_unit(0, 0, tg="sB")
        for _t in range(CT):     # xn8 column halves 512:1024 (qc1 + k-tiles 4-7)
            xn8_half1(_t)
        it0_fillers = {
            0: [lambda: qk_unit(5, 0, tg="mm")],
            1: [lambda: qk_unit(1, 0, tg="mm")],
            2: [lambda: qk_unit(4, 1, tg="mm"), lambda: v_unit(0, tg="mm")],
            3: [lambda: v_unit(1, tg="mm")],
            4: [lambda: v_unit(2, tg="mm")],
            5: [lambda: v_unit(3, tg="mm"), lambda: v_unit(4, tg="mm")],
            6: [lambda: v_unit(5, tg="mm"), lambda: v_unit(6, tg="mm")],
            7: [lambda: v_unit(7, tg="mm")],
        }

        # remaining QKV emitted as fillers inside the attention loop. Order
        # guarantees pair p+1's qkT units are emitted during iteration p
        # (3 filler slots/iteration), before any score matmul reads them;
        # the qc1 q-chunks land during the last qc0 iteration.
        def qk_filler(ot, qc):
            return lambda: qk_unit(ot, qc, tg="mm")
        fillers = [qk_filler(5, 1), qk_filler(6, 0), qk_filler(6, 1), qk_filler(2, 0),
                   qk_filler(0, 1), qk_filler(7, 0), qk_filler(7, 1), qk_filler(3, 0),
                   qk_filler(1, 1), qk_filler(2, 1), qk_filler(3, 1)]

        # ---- Phase 3: attention, exp-rate-limited ----
        expE = [pb.tile([128, 2, NT, 512], fp8, tag=f"expE{i}", name=f"expE{i}")
                for i in range(2)]
        attn8 = [pb.tile([128, 2, L], fp8, tag=f"attn8{u}", name=f"attn8{u}") for u in range(2)]
        out_sb = [pb.tile([128, CT, C], bf16, tag=f"os{qc}", name=f"os{qc}") for qc in range(QC)]

        def proj_unit(qc, t, tg="mm"):
            qt = 4 * qc + t
            po = ps.tile([128, 512], f32, tag=tg, name="po", bufs=2 if tg == "mm" else 1)
            for u in range(2):
                nc.tensor.matmul(po[:], attn8[u][:, :, 128 * qt:128 * (qt + 1)], wp(u),
                                 start=(u == 0), stop=(u == 1), perf_mode=DR)
            nc.vector.scalar_tensor_tensor(out_sb[qc][:, t, :], po[:], 1.0 / (WS * AS),
                                           xm[:, qt, :], Alu.mult, Alu.add)
            nc.sync.dma_start(out_d[:, qt:qt + 1, :], out_sb[qc][:, t:t + 1, :])

        def make_av_mm(avj, eb, p):
            def av_mm(u, j):
                nc.tensor.matmul(avj[j][:],
                                 v8[u][:, :, 192 * p + 64 * j:192 * p + 64 * j + 128],
                                 eb[:, j, 2 * u:2 * u + 2, :],
                                 start=(u == 0), stop=(u == 3), perf_mode=DR)
            return av_mm

        # qmv broadcast to 64 partitions once at startup; per-iteration
        # normalize then uses the denominator rows the AV matmuls replicated
        # 64x, so no gpsimd broadcast sits in the av-bank reuse path.
        qmv64 = pb.tile([64, L], f32, tag="qmv64", name="qmv64")
        nc.gpsimd.partition_broadcast(qmv64[:], qmv[:])

        def make_finalize(avj, p, qs):
            def finalize(j):
                dn = avj[j][64:128, :] if j == 0 else avj[j][0:64, :]
                dnv = pb.tile([64, 512], f32, tag="dnv", name="dnv", bufs=2)
                # rec = AS/denom; 1e30 on padded q columns -> rec ~ 0
                nc.vector.scalar_tensor_tensor(dnv[:], dn, 1.0 / AS,
                                               qmv64[:, qs], Alu.mult, Alu.mult)
                rec = pb.tile([64, 512], f32, tag="rec", name="rec", bufs=2)
                nc.vector.reciprocal_approx_fast(rec[:], dnv[:])
                nc.vector.tensor_tensor(attn8[p // 2][64 * j:64 * (j + 1), p % 2, qs],
                                        avj[j][64 * j:64 * (j + 1), :], rec[:], Alu.mult)
            return finalize

        # One heavy PE op (AV matmul / QKV filler) is placed between
        # consecutive score groups so the exp stream never waits long for its
        # scores. The last three AV matmuls and the softmax-normalize of
        # iteration i spill into iteration i+1's early slots.
        pending = []   # deferred closures from the previous iteration
        it = 0
        for qc in range(QC):
            qs = slice(512 * qc, 512 * (qc + 1))
            if qc == 1:
                # qc0 projection drains through the filler slots of qc1
                def proj_filler(t):
                    return lambda: proj_unit(0, t)
                fillers += [proj_filler(t) for t in range(CT)]
            for p in range(CT):
                eb = expE[it % 2]
                kT, qT = qkT[4 + p], qkT[p]
                # avj[0]: rows 0:64 = head 2p AV, rows 64:128 = denom (replicated)
                # avj[1]: rows 0:64 = denom, rows 64:128 = head 2p+1 AV
                # 3-way av-bank rotation: reuse period 1.5 iterations, so the
                # finalize reads never block the next iteration's AV writes
                avj = [ps.tile([128, 512], f32, tag=f"av{j}",
                               name=f"av{j}") for j in range(2)]
                av_mm = make_av_mm(avj, eb, p)

                # Each slot emits its scores+exp FIRST (so the exp stream is
                # never queued behind heavy PE work), then one heavy PE op:
                # slots 0-2 drain the previous iteration's tail, 3-7 run this
                # iteration's AV as its exps land; QKV/proj fillers ride along.
                for kt in range(NT):
                    s01 = ps.tile([128, 2, 512], f32, tag=("sA" if kt % 2 == 0 else "sB"),
                                  name="s01")
                    for j in range(2):
                        nc.tensor.matmul(s01[:, j, :],
                                         kT[64 * j:64 * (j + 1), 128 * kt:128 * (kt + 1)],
                                         qT[64 * j:64 * (j + 1), qs],
                                         start=True, stop=True)
                    if kt in DVE_EXP_SLOTS:
                        # Schraudolph: fp8e4m3 bits of exp(s/8) ~ s*log2e + 56
                        # (constant rounding bias cancels in softmax)
                        nc.vector.tensor_scalar(eb[:, :, kt, :].bitcast(i8),
                                                s01[:], LOG2E, 56.0,
                                                Alu.mult, Alu.add)
                    else:
                        nc.scalar.activation(eb[:, :, kt, :], s01[:], Act.Exp,
                                             bias=0.0, scale=0.125)
                    if it == 0:
                        for f in it0_fillers.pop(kt, []):
                            f()
                    if kt <= 2 and pending:
                        pending.pop(0)()
                    if kt == 3:
                        av_mm(0, 0)
                    elif kt == 4:
                        av_mm(0, 1)
                    elif kt == 5:
                        av_mm(1, 0)
                    elif kt == 6:
                        av_mm(1, 1)
                    elif kt == 7:
                        av_mm(2, 0)
                    if it != 0 and kt in (2, 4, 6):
                        for _ in range(1 if kt == 2 else 2):
                            if fillers:
                                fillers.pop(0)()
                fin = make_finalize(avj, p, qs)
                pending = [lambda f=av_mm: f(2, 1),
                           lambda f=av_mm, g=fin: (f(3, 0), g(0)),
                           lambda f=av_mm, g=fin: (f(3, 1), g(1))]
                it += 1

        while pending:
            pending.pop(0)()
        for t in range(CT):
            proj_unit(1, t, tg=("mm", "sA", "sB", "mm")[t])

    nc.compile()
    return nc


def _get_nc():
    if "nc" not in _CACHE:
        _CACHE["nc"] = _build()
    return _CACHE["nc"]


def _prep_weights(gamma, beta, Wqkv, bqkv, Wproj, bproj):
    """Host-side constant prep shared across cores."""
    W = np.asarray(Wqkv, np.float32)
    bq = np.asarray(bqkv, np.float32) + np.asarray(beta, np.float32) @ W   # fold beta
    Wp = np.asarray(Wproj, np.float32)
    bv = bq[2 * C:3 * C]
    # residual-side constant: bproj + bv @ Wproj (added to masked rows on host)
    resid_bias = np.asarray(bproj, np.float32) + bv @ Wp

    # w8A: [128, 6144] = [wqk pair0 | wqk pair1 | wv pair0 | wv pair1]
    # wqk[pair][p, t, ot*128+m] = WS * W[256*pair + 128*t + p, qk-cout(ot, m)]
    w8A = np.zeros((128, 6144), np.float32)
    Wqk = W[:, 0:2 * C]        # q couts 0:512, k couts 512:1024
    Wv = W[:, 2 * C:3 * C]
    for pair in range(2):
        for t in range(2):
            rows = slice(256 * pair + 128 * t, 256 * pair + 128 * (t + 1))
            w8A[:, 2048 * pair + 1024 * t:2048 * pair + 1024 * (t + 1)] = Wqk[rows, :]
            w8A[:, 4096 + 1024 * pair + 512 * t:4096 + 1024 * pair + 512 * (t + 1)] = Wv[rows, :]
    w8A = (w8A * WS).astype(FP8)

    # w8B: [128, 2048] = [wp u0 | wp u1]; wp[u][p, t, c] = WS * Wproj[(2u+t)*128 + p, c]
    w8B = np.zeros((128, 2048), np.float32)
    for u in range(2):
        for t in range(2):
            rows = slice(128 * (2 * u + t), 128 * (2 * u + t + 1))
            w8B[:, 1024 * u + 512 * t:1024 * u + 512 * (t + 1)] = Wp[rows, :]
    w8B = (w8B * WS).astype(FP8)

    # cA: [128, 28 + 4*32] f32 = [bqk 0:8 | gam 8:12 | vmsk 12:20 | vraw 20:28 | sel]
    cA = np.zeros((128, 28 + CT * G), np.float32)
    for ot in range(8):
        cA[:, ot] = bq[128 * ot:128 * (ot + 1)]
    cA[:, 8:12] = np.asarray(gamma, np.float32).reshape(CT, 128).T
    for t in range(CT):
        for c in range(128):
            cA[c, 28 + G * t + (128 * t + c) // CPG] = 1.0

    sT = np.zeros((G, C + 1), np.float32)
    for c in range(C):
        sT[c // CPG, c] = 1.0
    return w8A, w8B, cA, sT, resid_bias


def kernel(x, lengths, gamma, beta, Wqkv, bqkv, Wproj, bproj):
    from concourse.bass_utils import run_bass_kernel_spmd

    x = np.asarray(x, np.float32)
    lengths = np.asarray(lengths).astype(np.int64)
    w8A, w8B, cA, sT, resid_bias = _prep_weights(gamma, beta, Wqkv, bqkv, Wproj, bproj)

    in_maps = []
    for s in range(B):
        ln = int(lengths[s])
        mf = (np.arange(L) < ln).astype(np.float32)
        xms = x[s] * mf[:, None]
        xmr = xms + mf[:, None] * resid_bias[None, :]
        # xmT [128, CT, L]: partition p, channel-tile t -> channel 128t+p
        xmT = np.ascontiguousarray(
            xms.T.reshape(CT, 128, L).transpose(1, 0, 2)).astype(BF16)
        # xm [128, NT, C]: partition p, token-tile t -> token 128t+p
        xmr8 = np.ascontiguousarray(
            xmr.reshape(NT, 128, C).transpose(1, 0, 2)).astype(BF16)
        cAs = cA.copy()
        cAs[:, 12:20] = mf.reshape(NT, 128).T / WS
        cAs[:, 20:28] = mf.reshape(NT, 128).T
        sTs = sT.copy()
        sTs[:, C] = 1.0 / max(ln * CPG, 1)
        qmv = np.where(mf > 0, np.float32(1.0), np.float32(1e30)).reshape(1, L)
        in_maps.append(dict(xmT=xmT, cA=cAs, sT=sTs, qmv=qmv, w8A=w8A,
                            w8B=w8B, xmr=xmr8))

    nc = _get_nc()
    res = run_bass_kernel_spmd(nc, in_maps, core_ids=list(range(B)))
    _CACHE["last_res"] = res
    # out [128, NT, C] -> [L, C]; padded rows zeroed on host
    out = np.stack([np.asarray(res.results[s]["out"]).transpose(1, 0, 2).reshape(L, C)
                    for s in range(B)], axis=0).astype(np.float32)
    for s in range(B):
        out[s, int(lengths[s]):] = 0.0
    return out


if __name__ == "__main__":
    rng = np.random.default_rng(0)
    x = rng.standard_normal((B, L, C), dtype=np.float32)
    lengths = rng.integers(L // 2, L + 1, size=(B,))
    gamma = np.ones(C, np.float32)
    beta = np.zeros(C, np.float32)
    Wqkv = (rng.standard_normal((C, 3 * C)) * 0.02).astype(np.float32)
    bqkv = np.zeros(3 * C, np.float32)
    Wproj = (rng.standard_normal((C, C)) * 0.02).astype(np.float32)
    bproj = np.zeros(C, np.float32)
    out = kernel(x=x, lengths=lengths, gamma=gamma, beta=beta, Wqkv=Wqkv,
                 bqkv=bqkv, Wproj=Wproj, bproj=bproj)
    print("out", out.shape, out.dtype, np.abs(out).max())


# revision 48
# speedup vs baseline: 1.0270x; 1.0270x over previous
"""Trainium2 Bass kernel for nn_AttentionBlock (masked GroupNorm + jagged full attention).

Contract: kernel(**inputs) takes FULL unsharded inputs (as in reference.setup_inputs())
and returns the FULL [8, 1024, 512] fp32 output. Data-parallel over batch:
sample b -> NeuronCore b (8 cores).

Design: exp-rate-limited software pipeline (~118us/core, vs 213us baseline).
The softmax exp (8.4M elements/core) on the Activation engine is the hard
floor (64 calls x ~1.1us); everything else hides under it:
  - All dense matmuls (QKV, V, AV, proj) in fp8e4m3 with DoubleRow perf mode
    (2 contraction tiles packed per pass). Weights scaled x64 on host for fp8
    range; attention outputs x32; rescaled on PSUM->SBUF copies. Scores stay
    bf16 with two heads packed into PE row-halves per 128-token k-tile.
  - AV lhsT uses a sliding window over [v_h0 |mask| v_h1 | ...]: each DoubleRow
    AV matmul (must write all 128 dst partitions) yields 64 rows of AV plus 64
    replicated rows of the masked softmax denominator for free; normalize is a
    3-op DVE chain with no partition broadcast.
  - One exp slot per iteration runs on DVE via the Schraudolph bit trick:
    int8(score*log2e + 56) bit-cast to fp8e4m3 approximates exp(score/8)
    (softmax cancels the constant rounding bias; the mantissa-linear noise is
    in-family with fp8 quantization). This balances ScalarE (~70us) and DVE
    (~70us) instead of ScalarE being the lone 78us ceiling.
  - PSUM: 2x2 banks score/exp ping-pong, 2 AV accumulators, 2 general banks.
  - GroupNorm stats split DVE(bn_stats)/ScalarE(Copy+Square accumulate);
    normalize split DVE/ScalarE-Identity. One act-table swap total.
  - Per k-tile slot: scores+exp emitted first, then one deferred heavy PE op
    (prev-iteration AV tail, this iteration's AV, or a QKV/proj filler), so
    the in-order PE never queues scores behind bulk work.
  - Batched DMAs; bf16 in/out (output upcast on host).
"""

import numpy as np
import ml_dtypes
from contextlib import ExitStack

B, L, C, G, H = 8, 1024, 512, 32, 8
DH = C // H          # 64
CPG = C // G         # 16
EPS = 1e-5
NT = L // 128        # 8 token tiles
CT = C // 128        # 4 channel tiles
QC = L // 512        # 2 query chunks
WS = 64.0            # fp8 weight scale
AS = 32.0            # fp8 attention-output scale
LOG2E = 1.4426950408889634
DVE_EXP_SLOTS = (4,)  # per-iteration kt slots whose exp runs on DVE

BF16 = ml_dtypes.bfloat16
FP8 = ml_dtypes.float8_e4m3

_CACHE = {}


def _build():
    import concourse.tile as tile
    from concourse import bacc, mybir

    f32 = mybir.dt.float32
    bf16 = mybir.dt.bfloat16
    fp8 = mybir.dt.float8e4
    i8 = mybir.dt.int8
    Alu = mybir.AluOpType
    Act = mybir.ActivationFunctionType
    DR = mybir.MatmulPerfMode.DoubleRow

    nc = bacc.Bacc("TRN2", target_bir_lowering=False)

    # ---- per-core DRAM inputs (host-prepped; all shaped to match SBUF) ----
    xmT_d = nc.dram_tensor("xmT", [128, CT, L], bf16, kind="ExternalInput")
    cA_d = nc.dram_tensor("cA", [128, 28 + CT * G], f32, kind="ExternalInput")
    sT_d = nc.dram_tensor("sT", [G, C + 1], f32, kind="ExternalInput")
    qmv_d = nc.dram_tensor("qmv", [1, L], f32, kind="ExternalInput")
    w8A_d = nc.dram_tensor("w8A", [128, 6144], fp8, kind="ExternalInput")
    w8B_d = nc.dram_tensor("w8B", [128, 2048], fp8, kind="ExternalInput")
    xm_d = nc.dram_tensor("xmr", [128, NT, C], bf16, kind="ExternalInput")
    out_d = nc.dram_tensor("out", [128, NT, C], bf16, kind="ExternalOutput")

    with tile.TileContext(nc) as tc, ExitStack() as ctx:
        pc = ctx.enter_context(tc.tile_pool(name="consts", bufs=1))
        pb = ctx.enter_context(tc.tile_pool(name="big", bufs=1))
        ps = ctx.enter_context(tc.tile_pool(name="psum", bufs=1, space="PSUM"))

        # ---- input loads (xmT first: it gates GroupNorm; chunk order matches
        # the DVE/ACT stats split so both engines start early) ----
        xmT = pb.tile([128, CT, L], bf16, tag="xmT", name="xmT")
        cA = pc.tile([128, 28 + CT * G], f32, tag="cA", name="cA")
        sT = pc.tile([G, C + 1], f32, tag="sT", name="sT")
        nc.sync.dma_start(xmT[:, 0, :], xmT_d[:, 0, :])
        nc.sync.dma_start(cA[:], cA_d[:])
        nc.sync.dma_start(xmT[:, 2, :], xmT_d[:, 2, :])
        nc.sync.dma_start(sT[:], sT_d[:])
        nc.sync.dma_start(xmT[:, 1, :], xmT_d[:, 1, :])
        nc.sync.dma_start(xmT[:, 3, :], xmT_d[:, 3, :])
        qmv = pc.tile([1, L], f32, tag="qmv", name="qmv")
        nc.sync.dma_start(qmv[:], qmv_d[:])
        w8A = pc.tile([128, 6144], fp8, tag="w8A", name="w8A")
        nc.sync.dma_start(w8A[:], w8A_d[:])
        w8B = pc.tile([128, 2048], fp8, tag="w8B", name="w8B")
        nc.sync.dma_start(w8B[:], w8B_d[:])
        xm = pb.tile([128, NT, C], bf16, tag="xm", name="xm")
        nc.sync.dma_start(xm[:], xm_d[:])

        bqk = cA[:, 0:8]              # per-cout-tile qk bias (beta folded)
        gam = cA[:, 8:12]             # gamma per channel-tile
        vmsk = cA[:, 12:20]           # token-validity/WS per k-tile (v scale)
        vraw = cA[:, 20:28]           # raw token-validity per k-tile (denom mask)
        def sel(t):                   # [128, G] channel->group one-hot
            return cA[:, 28 + G * t:28 + G * (t + 1)]
        selT = sT[:, 0:C]             # [G, C] group->channel one-hot
        icnt = sT[:, C:C + 1]         # [G, 1] 1/(len*cpg)

        def wqk(pair, ot):            # lhsT [128, 2, 128] fp8 (q/k couts)
            a = w8A[:, 2048 * pair:2048 * (pair + 1)]
            return a.rearrange("p (t m) -> p t m", t=2)[:, :, 128 * ot:128 * (ot + 1)]

        def wv(pair):                 # rhs [128, 2, 512] fp8 (v couts)
            a = w8A[:, 4096 + 1024 * pair:4096 + 1024 * (pair + 1)]
            return a.rearrange("p (t m) -> p t m", t=2)

        def wp(u):                    # rhs [128, 2, 512] fp8 (proj couts)
            a = w8B[:, 1024 * u:1024 * (u + 1)]
            return a.rearrange("p (t m) -> p t m", t=2)

        # ---- Phase 1: GroupNorm (stats over valid tokens; zeros from host masking)
        # Stats split across engines: tiles 0,1 via DVE bn_stats; tiles 2,3 via
        # ScalarE Copy/Square with free-dim accumulate. All ScalarE functions
        # used in this kernel (Copy/Square/Identity/Exp) live in one act table,
        # so there are no mid-kernel table swaps.
        smm = [pb.tile([128, 2], f32, tag=f"smm{t}", name=f"smm{t}") for t in range(CT)]
        scr = pb.tile([128, L], f32, tag="scr", name="scr")
        # dummy Sqrt first: steers the initial act-table load to the set that
        # also holds Copy/Square/Identity, so only one swap (to Exp) remains
        dum = pb.tile([1, 1], f32, tag="dum", name="dum")
        nc.vector.memset(dum[:], 1.0)
        nc.scalar.activation(dum[:], dum[:], Act.Sqrt, bias=0.0, scale=1.0)
        for t in range(2):
            bns = pb.tile([128, 2, 6], f32, tag="bns", name="bns")
            nc.vector.bn_stats(bns[:, 0, :], xmT[:, t, 0:512])
            nc.vector.bn_stats(bns[:, 1, :], xmT[:, t, 512:1024])
            mv = pb.tile([128, 2], f32, tag="mv", name="mv")
            nc.vector.bn_aggr(mv[:], bns[:])
            sq = pb.tile([128, 1], f32, tag="sq", name="sq")
            nc.vector.tensor_mul(sq[:], mv[:, 0:1], mv[:, 0:1])
            # smm = [sum(x), sum(x^2)] recovered from mean/var over all 1024 (incl. zeros)
            nc.vector.tensor_scalar(smm[t][:, 0:1], mv[:, 0:1], float(L), None, Alu.mult)
            nc.vector.tensor_scalar(smm[t][:, 1:2], mv[:, 1:2], sq[:, 0:1], float(L), Alu.add, Alu.mult)
        for t in range(2, CT):
            nc.scalar.activation(scr[:], xmT[:, t, :], Act.Copy, bias=0.0,
                                 accum_out=smm[t][:, 0:1])
            nc.scalar.activation(scr[:], xmT[:, t, :], Act.Square,
                                 accum_out=smm[t][:, 1:2])
        ps_g = ps.tile([G, 2], f32, tag="mm", name="psg", bufs=2)
        for t in range(CT):
            nc.tensor.matmul(ps_g[:], sel(t), smm[t][:], start=(t == 0), stop=(t == CT - 1))
        grp = pb.tile([G, 2], f32, tag="grp", name="grp")      # [mean_g, rstd_g]
        me2 = pb.tile([G, 2], f32, tag="me2", name="me2")      # [mean_g, E[x^2]_g]
        nc.vector.tensor_scalar(me2[:], ps_g[:], icnt, None, Alu.mult)
        nc.vector.tensor_copy(grp[:, 0:1], me2[:, 0:1])
        ex2 = me2[:, 1:2]
        mm2 = pb.tile([G, 1], f32, tag="mm2", name="mm2")
        nc.vector.tensor_mul(mm2[:], me2[:, 0:1], me2[:, 0:1])
        varep = pb.tile([G, 1], f32, tag="veps", name="veps")
        nc.vector.scalar_tensor_tensor(varep[:], ex2[:], EPS, mm2[:], Alu.add, Alu.subtract)
        sd = pb.tile([G, 1], f32, tag="sd", name="sd")
        nc.scalar.activation(sd[:], varep[:], Act.Sqrt, bias=0.0, scale=1.0)
        nc.vector.reciprocal(grp[:, 1:2], sd[:])

        # xn in fp8, channel-pair-packed for DoubleRow: xn8[pair][p, t, l].
        # All four tiles normalized on DVE: the two ScalarE-Identity variants
        # queued behind ScalarE's stats until ~16.7us and gated the first exp;
        # on DVE they finish ~13us and the act-table swap to Exp happens right
        # after Sqrt(sd), off the critical path.
        xn8 = [pb.tile([128, 2, L], fp8, tag=f"xn8{u}", name=f"xn8{u}") for u in range(2)]
        rg = pb.tile([128, CT], f32, tag="rg", name="rg")
        for t in range(CT):
            ps_b = ps.tile([128, 2], f32, tag="mm", name="psb", bufs=2)
            nc.tensor.matmul(ps_b[:], selT[:, 128 * t:128 * (t + 1)], grp[:], start=True, stop=True)
            nc.vector.tensor_mul(rg[:, t:t + 1], ps_b[:, 1:2], gam[:, t:t + 1])
            nc.vector.tensor_scalar(xn8[t // 2][:, t % 2, :], xmT[:, t, :], ps_b[:, 0:1],
                                    rg[:, t:t + 1], Alu.subtract, Alu.mult)

        # ---- Phase 2: QKV in fp8 DoubleRow ----
        # v8[u] per k-tile-slot t: [v_h0 | mask | v_h1 | v_h2 | mask | v_h3 | ...]
        # (192 cols per head pair). The AV matmul for head 2p uses the window
        # [192p, 192p+128) = [v | mask], head 2p+1 uses [192p+64, 192p+192) =
        # [mask | v]: a DoubleRow matmul must write all 128 dst partitions, so
        # the mask half computes the softmax denominator in the spare rows.
        qkT = [pb.tile([128, L], bf16, tag=f"qkT{ot}", name=f"qkT{ot}") for ot in range(8)]
        v8 = [pb.tile([128, 2, 768], fp8, tag=f"v8{u}", name=f"v8{u}") for u in range(4)]
        psum_cycle = ["sA", "sB", "mm"]
        pcount = [0]

        def qk_unit(ot, qc, tg=None):
            if tg is None:
                tg = psum_cycle[pcount[0] % 3]; pcount[0] += 1
            pq = ps.tile([128, 512], f32, tag=tg, name="pq", bufs=2 if tg == "mm" else 1)
            for pr in range(2):
                nc.tensor.matmul(pq[:], wqk(pr, ot), xn8[pr][:, :, 512 * qc:512 * (qc + 1)],
                                 start=(pr == 0), stop=(pr == 1), perf_mode=DR)
            # qkT = pq/WS + bias (bias has beta folded in)
            nc.vector.tensor_scalar(qkT[ot][:, 512 * qc:512 * (qc + 1)], pq[:],
                                    1.0 / WS, bqk[:, ot:ot + 1], Alu.mult, Alu.add)

        def v_unit(kt, tg=None):
            if tg is None:
                tg = psum_cycle[pcount[0] % 3]; pcount[0] += 1
            pv = ps.tile([128, 512], f32, tag=tg, name="pv", bufs=2 if tg == "mm" else 1)
            for pr in range(2):
                nc.tensor.matmul(pv[:], xn8[pr][:, :, 128 * kt:128 * (kt + 1)], wv(pr),
                                 start=(pr == 0), stop=(pr == 1), perf_mode=DR)
            # v8 = pv * mf/WS  (padded-token rows zeroed); one strided DVE copy
            # into the [v_h0 | _ | v_h1] block structure (ACT stays exp-only)
            vt3 = v8[kt // 2][:, kt % 2, :].rearrange("p (a b d) -> p a b d", a=4, b=3)
            pvh = pv[:].rearrange("p (a j d) -> p a j d", a=4, j=2)
            nc.vector.tensor_scalar(vt3[:, :, 0:3:2, :], pvh[:], vmsk[:, kt:kt + 1],
                                    None, Alu.mult)
            # denominator mask columns (exact 1.0/0.0 in fp8)
            nc.gpsimd.tensor_copy(vt3[:, :, 1, :],
                                  vraw[:, kt:kt + 1].to_broadcast((128, 4, 64)))

        # Upfront: only what the first iterations' scores need. Everything
        # else (v units, remaining qk) drains through iteration-0 filler slots
        # so the first score matmuls aren't queued behind it on the in-order PE.
        qk_unit(4, 0, tg="sA")   # the s01(kt0) sA WAR is exactly this copy
        qk_unit(0, 0, tg="sB")
        it0_fillers = {
            0: [lambda: qk_unit(5, 0, tg="mm")],
            1: [lambda: qk_unit(1, 0, tg="mm")],
            2: [lambda: qk_unit(4, 1, tg="mm"), lambda: v_unit(0, tg="mm")],
            3: [lambda: v_unit(1, tg="mm")],
            4: [lambda: v_unit(2, tg="mm")],
            5: [lambda: v_unit(3, tg="mm"), lambda: v_unit(4, tg="mm")],
            6: [lambda: v_unit(5, tg="mm"), lambda: v_unit(6, tg="mm")],
            7: [lambda: v_unit(7, tg="mm")],
        }

        # remaining QKV emitted as fillers inside the attention loop. Order
        # guarantees pair p+1's qkT units are emitted during iteration p
        # (3 filler slots/iteration), before any score matmul reads them;
        # the qc1 q-chunks land during the last qc0 iteration.
        def qk_filler(ot, qc):
            return lambda: qk_unit(ot, qc, tg="mm")
        fillers = [qk_filler(5, 1), qk_filler(6, 0), qk_filler(6, 1), qk_filler(2, 0),
                   qk_filler(0, 1), qk_filler(7, 0), qk_filler(7, 1), qk_filler(3, 0),
                   qk_filler(1, 1), qk_filler(2, 1), qk_filler(3, 1)]

        # ---- Phase 3: attention, exp-rate-limited ----
        expE = [pb.tile([128, 2, NT, 512], fp8, tag=f"expE{i}", name=f"expE{i}")
                for i in range(2)]
        attn8 = [pb.tile([128, 2, L], fp8, tag=f"attn8{u}", name=f"attn8{u}") for u in range(2)]
        out_sb = [pb.tile([128, CT, C], bf16, tag=f"os{qc}", name=f"os{qc}") for qc in range(QC)]

        def proj_unit(qc, t, tg="mm"):
            qt = 4 * qc + t
            po = ps.tile([128, 512], f32, tag=tg, name="po", bufs=2 if tg == "mm" else 1)
            for u in range(2):
                nc.tensor.matmul(po[:], attn8[u][:, :, 128 * qt:128 * (qt + 1)], wp(u),
                                 start=(u == 0), stop=(u == 1), perf_mode=DR)
            nc.vector.scalar_tensor_tensor(out_sb[qc][:, t, :], po[:], 1.0 / (WS * AS),
                                           xm[:, qt, :], Alu.mult, Alu.add)
            nc.sync.dma_start(out_d[:, qt:qt + 1, :], out_sb[qc][:, t:t + 1, :])

        def make_av_mm(avj, eb, p):
            def av_mm(u, j):
                nc.tensor.matmul(avj[j][:],
                                 v8[u][:, :, 192 * p + 64 * j:192 * p + 64 * j + 128],
                                 eb[:, j, 2 * u:2 * u + 2, :],
                                 start=(u == 0), stop=(u == 3), perf_mode=DR)
            return av_mm

        # qmv broadcast to 64 partitions once at startup; per-iteration
        # normalize then uses the denominator rows the AV matmuls replicated
        # 64x, so no gpsimd broadcast sits in the av-bank reuse path.
        qmv64 = pb.tile([64, L], f32, tag="qmv64", name="qmv64")
        nc.gpsimd.partition_broadcast(qmv64[:], qmv[:])

        def make_finalize(avj, p, qs):
            def finalize(j):
                dn = avj[j][64:128, :] if j == 0 else avj[j][0:64, :]
                dnv = pb.tile([64, 512], f32, tag="dnv", name="dnv", bufs=2)
                # rec = AS/denom; 1e30 on padded q columns -> rec ~ 0
                nc.vector.scalar_tensor_tensor(dnv[:], dn, 1.0 / AS,
                                               qmv64[:, qs], Alu.mult, Alu.mult)
                rec = pb.tile([64, 512], f32, tag="rec", name="rec", bufs=2)
                nc.vector.reciprocal_approx_fast(rec[:], dnv[:])
                nc.vector.tensor_tensor(attn8[p // 2][64 * j:64 * (j + 1), p % 2, qs],
                                        avj[j][64 * j:64 * (j + 1), :], rec[:], Alu.mult)
            return finalize

        # One heavy PE op (AV matmul / QKV filler) is placed between
        # consecutive score groups so the exp stream never waits long for its
        # scores. The last three AV matmuls and the softmax-normalize of
        # iteration i spill into iteration i+1's early slots.
        pending = []   # deferred closures from the previous iteration
        it = 0
        for qc in range(QC):
            qs = slice(512 * qc, 512 * (qc + 1))
            if qc == 1:
                # qc0 projection drains through the filler slots of qc1
                def proj_filler(t):
                    return lambda: proj_unit(0, t)
                fillers += [proj_filler(t) for t in range(CT)]
            for p in range(CT):
                eb = expE[it % 2]
                kT, qT = qkT[4 + p], qkT[p]
                # avj[0]: rows 0:64 = head 2p AV, rows 64:128 = denom (replicated)
                # avj[1]: rows 0:64 = denom, rows 64:128 = head 2p+1 AV
                # 3-way av-bank rotation: reuse period 1.5 iterations, so the
                # finalize reads never block the next iteration's AV writes
                avj = [ps.tile([128, 512], f32, tag=f"av{j}",
                               name=f"av{j}") for j in range(2)]
                av_mm = make_av_mm(avj, eb, p)

                # Each slot emits its scores+exp FIRST (so the exp stream is
                # never queued behind heavy PE work), then one heavy PE op:
                # slots 0-2 drain the previous iteration's tail, 3-7 run this
                # iteration's AV as its exps land; QKV/proj fillers ride along.
                for kt in range(NT):
                    s01 = ps.tile([128, 2, 512], f32, tag=("sA" if kt % 2 == 0 else "sB"),
                                  name="s01")
                    for j in range(2):
                        nc.tensor.matmul(s01[:, j, :],
                                         kT[64 * j:64 * (j + 1), 128 * kt:128 * (kt + 1)],
                                         qT[64 * j:64 * (j + 1), qs],
                                         start=True, stop=True)
                    if kt in DVE_EXP_SLOTS:
                        # Schraudolph: fp8e4m3 bits of exp(s/8) ~ s*log2e + 56
                        # (constant rounding bias cancels in softmax)
                        nc.vector.tensor_scalar(eb[:, :, kt, :].bitcast(i8),
                                                s01[:], LOG2E, 56.0,
                                                Alu.mult, Alu.add)
                    else:
                        nc.scalar.activation(eb[:, :, kt, :], s01[:], Act.Exp,
                                             bias=0.0, scale=0.125)
                    if it == 0:
                        for f in it0_fillers.pop(kt, []):
                            f()
                    if kt <= 2 and pending:
                        pending.pop(0)()
                    if kt == 3:
                        av_mm(0, 0)
                    elif kt == 4:
                        av_mm(0, 1)
                    elif kt == 5:
                        av_mm(1, 0)
                    elif kt == 6:
                        av_mm(1, 1)
                    elif kt == 7:
                        av_mm(2, 0)
                    if it != 0 and kt in (2, 4, 6):
                        for _ in range(1 if kt == 2 else 2):
                            if fillers:
                                fillers.pop(0)()
                fin = make_finalize(avj, p, qs)
                pending = [lambda f=av_mm: f(2, 1),
                           lambda f=av_mm, g=fin: (f(3, 0), g(0)),
                           lambda f=av_mm, g=fin: (f(3, 1), g(1))]
                it += 1

        while pending:
            pending.pop(0)()
        for t in range(CT):
            proj_unit(1, t, tg=("mm", "sA", "sB", "mm")[t])

    nc.compile()
    return nc


def _get_nc():
    if "nc" not in _CACHE:
        _CACHE["nc"] = _build()
    return _CACHE["nc"]


def _prep_weights(gamma, beta, Wqkv, bqkv, Wproj, bproj):
    """Host-side constant prep shared across cores."""
    W = np.asarray(Wqkv, np.float32)
    bq = np.asarray(bqkv, np.float32) + np.asarray(beta, np.float32) @ W   # fold beta
    Wp = np.asarray(Wproj, np.float32)
    bv = bq[2 * C:3 * C]
    # residual-side constant: bproj + bv @ Wproj (added to masked rows on host)
    resid_bias = np.asarray(bproj, np.float32) + bv @ Wp

    # w8A: [128, 6144] = [wqk pair0 | wqk pair1 | wv pair0 | wv pair1]
    # wqk[pair][p, t, ot*128+m] = WS * W[256*pair + 128*t + p, qk-cout(ot, m)]
    w8A = np.zeros((128, 6144), np.float32)
    Wqk = W[:, 0:2 * C]        # q couts 0:512, k couts 512:1024
    Wv = W[:, 2 * C:3 * C]
    for pair in range(2):
        for t in range(2):
            rows = slice(256 * pair + 128 * t, 256 * pair + 128 * (t + 1))
            w8A[:, 2048 * pair + 1024 * t:2048 * pair + 1024 * (t + 1)] = Wqk[rows, :]
            w8A[:, 4096 + 1024 * pair + 512 * t:4096 + 1024 * pair + 512 * (t + 1)] = Wv[rows, :]
    w8A = (w8A * WS).astype(FP8)

    # w8B: [128, 2048] = [wp u0 | wp u1]; wp[u][p, t, c] = WS * Wproj[(2u+t)*128 + p, c]
    w8B = np.zeros((128, 2048), np.float32)
    for u in range(2):
        for t in range(2):
            rows = slice(128 * (2 * u + t), 128 * (2 * u + t + 1))
            w8B[:, 1024 * u + 512 * t:1024 * u + 512 * (t + 1)] = Wp[rows, :]
    w8B = (w8B * WS).astype(FP8)

    # cA: [128, 28 + 4*32] f32 = [bqk 0:8 | gam 8:12 | vmsk 12:20 | vraw 20:28 | sel]
    cA = np.zeros((128, 28 + CT * G), np.float32)
    for ot in range(8):
        cA[:, ot] = bq[128 * ot:128 * (ot + 1)]
    cA[:, 8:12] = np.asarray(gamma, np.float32).reshape(CT, 128).T
    for t in range(CT):
        for c in range(128):
            cA[c, 28 + G * t + (128 * t + c) // CPG] = 1.0

    sT = np.zeros((G, C + 1), np.float32)
    for c in range(C):
        sT[c // CPG, c] = 1.0
    return w8A, w8B, cA, sT, resid_bias


def kernel(x, lengths, gamma, beta, Wqkv, bqkv, Wproj, bproj):
    from concourse.bass_utils import run_bass_kernel_spmd

    x = np.asarray(x, np.float32)
    lengths = np.asarray(lengths).astype(np.int64)
    w8A, w8B, cA, sT, resid_bias = _prep_weights(gamma, beta, Wqkv, bqkv, Wproj, bproj)

    in_maps = []
    for s in range(B):
        ln = int(lengths[s])
        mf = (np.arange(L) < ln).astype(np.float32)
        xms = x[s] * mf[:, None]
        xmr = xms + mf[:, None] * resid_bias[None, :]
        # xmT [128, CT, L]: partition p, channel-tile t -> channel 128t+p
        xmT = np.ascontiguousarray(
            xms.T.reshape(CT, 128, L).transpose(1, 0, 2)).astype(BF16)
        # xm [128, NT, C]: partition p, token-tile t -> token 128t+p
        xmr8 = np.ascontiguousarray(
            xmr.reshape(NT, 128, C).transpose(1, 0, 2)).astype(BF16)
        cAs = cA.copy()
        cAs[:, 12:20] = mf.reshape(NT, 128).T / WS
        cAs[:, 20:28] = mf.reshape(NT, 128).T
        sTs = sT.copy()
        sTs[:, C] = 1.0 / max(ln * CPG, 1)
        qmv = np.where(mf > 0, np.float32(1.0), np.float32(1e30)).reshape(1, L)
        in_maps.append(dict(xmT=xmT, cA=cAs, sT=sTs, qmv=qmv, w8A=w8A,
                            w8B=w8B, xmr=xmr8))

    nc = _get_nc()
    res = run_bass_kernel_spmd(nc, in_maps, core_ids=list(range(B)))
    _CACHE["last_res"] = res
    # out [128, NT, C] -> [L, C]; padded rows zeroed on host
    out = np.stack([np.asarray(res.results[s]["out"]).transpose(1, 0, 2).reshape(L, C)
                    for s in range(B)], axis=0).astype(np.float32)
    for s in range(B):
        out[s, int(lengths[s]):] = 0.0
    return out


if __name__ == "__main__":
    rng = np.random.default_rng(0)
    x = rng.standard_normal((B, L, C), dtype=np.float32)
    lengths = rng.integers(L // 2, L + 1, size=(B,))
    gamma = np.ones(C, np.float32)
    beta = np.zeros(C, np.float32)
    Wqkv = (rng.standard_normal((C, 3 * C)) * 0.02).astype(np.float32)
    bqkv = np.zeros(3 * C, np.float32)
    Wproj = (rng.standard_normal((C, C)) * 0.02).astype(np.float32)
    bproj = np.zeros(C, np.float32)
    out = kernel(x=x, lengths=lengths, gamma=gamma, beta=beta, Wqkv=Wqkv,
                 bqkv=bqkv, Wproj=Wproj, bproj=bproj)
    print("out", out.shape, out.dtype, np.abs(out).max())


# revision 49
# speedup vs baseline: 1.0341x; 1.0069x over previous
"""Trainium2 Bass kernel for nn_AttentionBlock (masked GroupNorm + jagged full attention).

Contract: kernel(**inputs) takes FULL unsharded inputs (as in reference.setup_inputs())
and returns the FULL [8, 1024, 512] fp32 output. Data-parallel over batch:
sample b -> NeuronCore b (8 cores).

Design: exp-rate-limited software pipeline (~118us/core, vs 213us baseline).
The softmax exp (8.4M elements/core) on the Activation engine is the hard
floor (64 calls x ~1.1us); everything else hides under it:
  - All dense matmuls (QKV, V, AV, proj) in fp8e4m3 with DoubleRow perf mode
    (2 contraction tiles packed per pass). Weights scaled x64 on host for fp8
    range; attention outputs x32; rescaled on PSUM->SBUF copies. Scores stay
    bf16 with two heads packed into PE row-halves per 128-token k-tile.
  - AV lhsT uses a sliding window over [v_h0 |mask| v_h1 | ...]: each DoubleRow
    AV matmul (must write all 128 dst partitions) yields 64 rows of AV plus 64
    replicated rows of the masked softmax denominator for free; normalize is a
    3-op DVE chain with no partition broadcast.
  - One exp slot per iteration runs on DVE via the Schraudolph bit trick:
    int8(score*log2e + 56) bit-cast to fp8e4m3 approximates exp(score/8)
    (softmax cancels the constant rounding bias; the mantissa-linear noise is
    in-family with fp8 quantization). This balances ScalarE (~70us) and DVE
    (~70us) instead of ScalarE being the lone 78us ceiling.
  - PSUM: 2x2 banks score/exp ping-pong, 2 AV accumulators, 2 general banks.
  - GroupNorm stats split DVE(bn_stats)/ScalarE(Copy+Square accumulate);
    normalize split DVE/ScalarE-Identity. One act-table swap total.
  - Per k-tile slot: scores+exp emitted first, then one deferred heavy PE op
    (prev-iteration AV tail, this iteration's AV, or a QKV/proj filler), so
    the in-order PE never queues scores behind bulk work.
  - Batched DMAs; bf16 in/out (output upcast on host).
"""

import numpy as np
import ml_dtypes
from contextlib import ExitStack

B, L, C, G, H = 8, 1024, 512, 32, 8
DH = C // H          # 64
CPG = C // G         # 16
EPS = 1e-5
NT = L // 128        # 8 token tiles
CT = C // 128        # 4 channel tiles
QC = L // 512        # 2 query chunks
WS = 64.0            # fp8 weight scale
AS = 32.0            # fp8 attention-output scale
LOG2E = 1.4426950408889634
DVE_EXP_SLOTS = (7,)  # per-iteration kt slots whose exp runs on DVE; slot 7's
                      # consumer (av_mm(3,1)) only runs at the next iteration's
                      # slot 2, so the DVE exp never stalls the in-order PE

BF16 = ml_dtypes.bfloat16
FP8 = ml_dtypes.float8_e4m3

_CACHE = {}


def _build():
    import concourse.tile as tile
    from concourse import bacc, mybir

    f32 = mybir.dt.float32
    bf16 = mybir.dt.bfloat16
    fp8 = mybir.dt.float8e4
    i8 = mybir.dt.int8
    Alu = mybir.AluOpType
    Act = mybir.ActivationFunctionType
    DR = mybir.MatmulPerfMode.DoubleRow

    nc = bacc.Bacc("TRN2", target_bir_lowering=False)

    # ---- per-core DRAM inputs (host-prepped; all shaped to match SBUF) ----
    xmT_d = nc.dram_tensor("xmT", [128, CT, L], bf16, kind="ExternalInput")
    cA_d = nc.dram_tensor("cA", [128, 28 + CT * G], f32, kind="ExternalInput")
    sT_d = nc.dram_tensor("sT", [G, C + 1], f32, kind="ExternalInput")
    qmv_d = nc.dram_tensor("qmv", [1, L], f32, kind="ExternalInput")
    w8A_d = nc.dram_tensor("w8A", [128, 6144], fp8, kind="ExternalInput")
    w8B_d = nc.dram_tensor("w8B", [128, 2048], fp8, kind="ExternalInput")
    xm_d = nc.dram_tensor("xmr", [128, NT, C], bf16, kind="ExternalInput")
    out_d = nc.dram_tensor("out", [128, NT, C], bf16, kind="ExternalOutput")

    with tile.TileContext(nc) as tc, ExitStack() as ctx:
        pc = ctx.enter_context(tc.tile_pool(name="consts", bufs=1))
        pb = ctx.enter_context(tc.tile_pool(name="big", bufs=1))
        ps = ctx.enter_context(tc.tile_pool(name="psum", bufs=1, space="PSUM"))

        # ---- input loads (xmT first: it gates GroupNorm; chunk order matches
        # the DVE/ACT stats split so both engines start early) ----
        xmT = pb.tile([128, CT, L], bf16, tag="xmT", name="xmT")
        cA = pc.tile([128, 28 + CT * G], f32, tag="cA", name="cA")
        sT = pc.tile([G, C + 1], f32, tag="sT", name="sT")
        nc.sync.dma_start(xmT[:, 0, :], xmT_d[:, 0, :])
        nc.sync.dma_start(cA[:], cA_d[:])
        nc.sync.dma_start(xmT[:, 2, :], xmT_d[:, 2, :])
        nc.sync.dma_start(sT[:], sT_d[:])
        nc.sync.dma_start(xmT[:, 1, :], xmT_d[:, 1, :])
        nc.sync.dma_start(xmT[:, 3, :], xmT_d[:, 3, :])
        qmv = pc.tile([1, L], f32, tag="qmv", name="qmv")
        nc.sync.dma_start(qmv[:], qmv_d[:])
        w8A = pc.tile([128, 6144], fp8, tag="w8A", name="w8A")
        nc.sync.dma_start(w8A[:], w8A_d[:])
        w8B = pc.tile([128, 2048], fp8, tag="w8B", name="w8B")
        nc.sync.dma_start(w8B[:], w8B_d[:])
        xm = pb.tile([128, NT, C], bf16, tag="xm", name="xm")
        nc.sync.dma_start(xm[:], xm_d[:])

        bqk = cA[:, 0:8]              # per-cout-tile qk bias (beta folded)
        gam = cA[:, 8:12]             # gamma per channel-tile
        vmsk = cA[:, 12:20]           # token-validity/WS per k-tile (v scale)
        vraw = cA[:, 20:28]           # raw token-validity per k-tile (denom mask)
        def sel(t):                   # [128, G] channel->group one-hot
            return cA[:, 28 + G * t:28 + G * (t + 1)]
        selT = sT[:, 0:C]             # [G, C] group->channel one-hot
        icnt = sT[:, C:C + 1]         # [G, 1] 1/(len*cpg)

        def wqk(pair, ot):            # lhsT [128, 2, 128] fp8 (q/k couts)
            a = w8A[:, 2048 * pair:2048 * (pair + 1)]
            return a.rearrange("p (t m) -> p t m", t=2)[:, :, 128 * ot:128 * (ot + 1)]

        def wv(pair):                 # rhs [128, 2, 512] fp8 (v couts)
            a = w8A[:, 4096 + 1024 * pair:4096 + 1024 * (pair + 1)]
            return a.rearrange("p (t m) -> p t m", t=2)

        def wp(u):                    # rhs [128, 2, 512] fp8 (proj couts)
            a = w8B[:, 1024 * u:1024 * (u + 1)]
            return a.rearrange("p (t m) -> p t m", t=2)

        # ---- Phase 1: GroupNorm (stats over valid tokens; zeros from host masking)
        # Stats split across engines: tiles 0,1 via DVE bn_stats; tiles 2,3 via
        # ScalarE Copy/Square with free-dim accumulate. All ScalarE functions
        # used in this kernel (Copy/Square/Identity/Exp) live in one act table,
        # so there are no mid-kernel table swaps.
        smm = [pb.tile([128, 2], f32, tag=f"smm{t}", name=f"smm{t}") for t in range(CT)]
        scr = pb.tile([128, L], f32, tag="scr", name="scr")
        # dummy Sqrt first: steers the initial act-table load to the set that
        # also holds Copy/Square/Identity, so only one swap (to Exp) remains
        dum = pb.tile([1, 1], f32, tag="dum", name="dum")
        nc.vector.memset(dum[:], 1.0)
        nc.scalar.activation(dum[:], dum[:], Act.Sqrt, bias=0.0, scale=1.0)
        for t in range(2):
            bns = pb.tile([128, 2, 6], f32, tag="bns", name="bns")
            nc.vector.bn_stats(bns[:, 0, :], xmT[:, t, 0:512])
            nc.vector.bn_stats(bns[:, 1, :], xmT[:, t, 512:1024])
            mv = pb.tile([128, 2], f32, tag="mv", name="mv")
            nc.vector.bn_aggr(mv[:], bns[:])
            sq = pb.tile([128, 1], f32, tag="sq", name="sq")
            nc.vector.tensor_mul(sq[:], mv[:, 0:1], mv[:, 0:1])
            # smm = [sum(x), sum(x^2)] recovered from mean/var over all 1024 (incl. zeros)
            nc.vector.tensor_scalar(smm[t][:, 0:1], mv[:, 0:1], float(L), None, Alu.mult)
            nc.vector.tensor_scalar(smm[t][:, 1:2], mv[:, 1:2], sq[:, 0:1], float(L), Alu.add, Alu.mult)
        for t in range(2, CT):
            nc.scalar.activation(scr[:], xmT[:, t, :], Act.Copy, bias=0.0,
                                 accum_out=smm[t][:, 0:1])
            nc.scalar.activation(scr[:], xmT[:, t, :], Act.Square,
                                 accum_out=smm[t][:, 1:2])
        ps_g = ps.tile([G, 2], f32, tag="mm", name="psg", bufs=2)
        for t in range(CT):
            nc.tensor.matmul(ps_g[:], sel(t), smm[t][:], start=(t == 0), stop=(t == CT - 1))
        grp = pb.tile([G, 2], f32, tag="grp", name="grp")      # [mean_g, rstd_g]
        me2 = pb.tile([G, 2], f32, tag="me2", name="me2")      # [mean_g, E[x^2]_g]
        nc.vector.tensor_scalar(me2[:], ps_g[:], icnt, None, Alu.mult)
        nc.vector.tensor_copy(grp[:, 0:1], me2[:, 0:1])
        ex2 = me2[:, 1:2]
        mm2 = pb.tile([G, 1], f32, tag="mm2", name="mm2")
        nc.vector.tensor_mul(mm2[:], me2[:, 0:1], me2[:, 0:1])
        varep = pb.tile([G, 1], f32, tag="veps", name="veps")
        nc.vector.scalar_tensor_tensor(varep[:], ex2[:], EPS, mm2[:], Alu.add, Alu.subtract)
        sd = pb.tile([G, 1], f32, tag="sd", name="sd")
        nc.scalar.activation(sd[:], varep[:], Act.Sqrt, bias=0.0, scale=1.0)
        nc.vector.reciprocal(grp[:, 1:2], sd[:])

        # xn in fp8, channel-pair-packed for DoubleRow: xn8[pair][p, t, l].
        # All four tiles normalized on DVE: the two ScalarE-Identity variants
        # queued behind ScalarE's stats until ~16.7us and gated the first exp;
        # on DVE they finish ~13us and the act-table swap to Exp happens right
        # after Sqrt(sd), off the critical path.
        xn8 = [pb.tile([128, 2, L], fp8, tag=f"xn8{u}", name=f"xn8{u}") for u in range(2)]
        rg = pb.tile([128, CT], f32, tag="rg", name="rg")
        for t in range(CT):
            ps_b = ps.tile([128, 2], f32, tag="mm", name="psb", bufs=2)
            nc.tensor.matmul(ps_b[:], selT[:, 128 * t:128 * (t + 1)], grp[:], start=True, stop=True)
            nc.vector.tensor_mul(rg[:, t:t + 1], ps_b[:, 1:2], gam[:, t:t + 1])
            nc.vector.tensor_scalar(xn8[t // 2][:, t % 2, :], xmT[:, t, :], ps_b[:, 0:1],
                                    rg[:, t:t + 1], Alu.subtract, Alu.mult)

        # ---- Phase 2: QKV in fp8 DoubleRow ----
        # v8[u] per k-tile-slot t: [v_h0 | mask | v_h1 | v_h2 | mask | v_h3 | ...]
        # (192 cols per head pair). The AV matmul for head 2p uses the window
        # [192p, 192p+128) = [v | mask], head 2p+1 uses [192p+64, 192p+192) =
        # [mask | v]: a DoubleRow matmul must write all 128 dst partitions, so
        # the mask half computes the softmax denominator in the spare rows.
        qkT = [pb.tile([128, L], bf16, tag=f"qkT{ot}", name=f"qkT{ot}") for ot in range(8)]
        v8 = [pb.tile([128, 2, 768], fp8, tag=f"v8{u}", name=f"v8{u}") for u in range(4)]
        psum_cycle = ["sA", "sB", "mm"]
        pcount = [0]

        def qk_unit(ot, qc, tg=None):
            if tg is None:
                tg = psum_cycle[pcount[0] % 3]; pcount[0] += 1
            pq = ps.tile([128, 512], f32, tag=tg, name="pq", bufs=2 if tg == "mm" else 1)
            for pr in range(2):
                nc.tensor.matmul(pq[:], wqk(pr, ot), xn8[pr][:, :, 512 * qc:512 * (qc + 1)],
                                 start=(pr == 0), stop=(pr == 1), perf_mode=DR)
            # qkT = pq/WS + bias (bias has beta folded in)
            nc.vector.tensor_scalar(qkT[ot][:, 512 * qc:512 * (qc + 1)], pq[:],
                                    1.0 / WS, bqk[:, ot:ot + 1], Alu.mult, Alu.add)

        def v_unit(kt, tg=None):
            if tg is None:
                tg = psum_cycle[pcount[0] % 3]; pcount[0] += 1
            pv = ps.tile([128, 512], f32, tag=tg, name="pv", bufs=2 if tg == "mm" else 1)
            for pr in range(2):
                nc.tensor.matmul(pv[:], xn8[pr][:, :, 128 * kt:128 * (kt + 1)], wv(pr),
                                 start=(pr == 0), stop=(pr == 1), perf_mode=DR)
            # v8 = pv * mf/WS  (padded-token rows zeroed); one strided DVE copy
            # into the [v_h0 | _ | v_h1] block structure (ACT stays exp-only)
            vt3 = v8[kt // 2][:, kt % 2, :].rearrange("p (a b d) -> p a b d", a=4, b=3)
            pvh = pv[:].rearrange("p (a j d) -> p a j d", a=4, j=2)
            nc.vector.tensor_scalar(vt3[:, :, 0:3:2, :], pvh[:], vmsk[:, kt:kt + 1],
                                    None, Alu.mult)
            # denominator mask columns (exact 1.0/0.0 in fp8)
            nc.gpsimd.tensor_copy(vt3[:, :, 1, :],
                                  vraw[:, kt:kt + 1].to_broadcast((128, 4, 64)))

        # Upfront: only what the first iterations' scores need. Everything
        # else (v units, remaining qk) drains through iteration-0 filler slots
        # so the first score matmuls aren't queued behind it on the in-order PE.
        qk_unit(4, 0, tg="sA")   # the s01(kt0) sA WAR is exactly this copy
        qk_unit(0, 0, tg="sB")
        it0_fillers = {
            0: [lambda: qk_unit(5, 0, tg="mm")],
            1: [lambda: qk_unit(1, 0, tg="mm")],
            2: [lambda: qk_unit(4, 1, tg="mm"), lambda: v_unit(0, tg="mm")],
            3: [lambda: v_unit(1, tg="mm")],
            4: [lambda: v_unit(2, tg="mm")],
            5: [lambda: v_unit(3, tg="mm"), lambda: v_unit(4, tg="mm")],
            6: [lambda: v_unit(5, tg="mm"), lambda: v_unit(6, tg="mm")],
            7: [lambda: v_unit(7, tg="mm")],
        }

        # remaining QKV emitted as fillers inside the attention loop. Order
        # guarantees pair p+1's qkT units are emitted during iteration p
        # (3 filler slots/iteration), before any score matmul reads them;
        # the qc1 q-chunks land during the last qc0 iteration.
        def qk_filler(ot, qc):
            return lambda: qk_unit(ot, qc, tg="mm")
        fillers = [qk_filler(5, 1), qk_filler(6, 0), qk_filler(6, 1), qk_filler(2, 0),
                   qk_filler(0, 1), qk_filler(7, 0), qk_filler(7, 1), qk_filler(3, 0),
                   qk_filler(1, 1), qk_filler(2, 1), qk_filler(3, 1)]

        # ---- Phase 3: attention, exp-rate-limited ----
        expE = [pb.tile([128, 2, NT, 512], fp8, tag=f"expE{i}", name=f"expE{i}")
                for i in range(2)]
        attn8 = [pb.tile([128, 2, L], fp8, tag=f"attn8{u}", name=f"attn8{u}") for u in range(2)]
        out_sb = [pb.tile([128, CT, C], bf16, tag=f"os{qc}", name=f"os{qc}") for qc in range(QC)]

        def proj_unit(qc, t, tg="mm"):
            qt = 4 * qc + t
            po = ps.tile([128, 512], f32, tag=tg, name="po", bufs=2 if tg == "mm" else 1)
            for u in range(2):
                nc.tensor.matmul(po[:], attn8[u][:, :, 128 * qt:128 * (qt + 1)], wp(u),
                                 start=(u == 0), stop=(u == 1), perf_mode=DR)
            nc.vector.scalar_tensor_tensor(out_sb[qc][:, t, :], po[:], 1.0 / (WS * AS),
                                           xm[:, qt, :], Alu.mult, Alu.add)
            nc.sync.dma_start(out_d[:, qt:qt + 1, :], out_sb[qc][:, t:t + 1, :])

        def make_av_mm(avj, eb, p):
            def av_mm(u, j):
                nc.tensor.matmul(avj[j][:],
                                 v8[u][:, :, 192 * p + 64 * j:192 * p + 64 * j + 128],
                                 eb[:, j, 2 * u:2 * u + 2, :],
                                 start=(u == 0), stop=(u == 3), perf_mode=DR)
            return av_mm

        # qmv broadcast to 64 partitions once at startup; per-iteration
        # normalize then uses the denominator rows the AV matmuls replicated
        # 64x, so no gpsimd broadcast sits in the av-bank reuse path.
        qmv64 = pb.tile([64, L], f32, tag="qmv64", name="qmv64")
        nc.gpsimd.partition_broadcast(qmv64[:], qmv[:])

        def make_finalize(avj, p, qs):
            def finalize(j):
                dn = avj[j][64:128, :] if j == 0 else avj[j][0:64, :]
                dnv = pb.tile([64, 512], f32, tag="dnv", name="dnv", bufs=2)
                # rec = AS/denom; 1e30 on padded q columns -> rec ~ 0
                nc.vector.scalar_tensor_tensor(dnv[:], dn, 1.0 / AS,
                                               qmv64[:, qs], Alu.mult, Alu.mult)
                rec = pb.tile([64, 512], f32, tag="rec", name="rec", bufs=2)
                nc.vector.reciprocal_approx_fast(rec[:], dnv[:])
                nc.vector.tensor_tensor(attn8[p // 2][64 * j:64 * (j + 1), p % 2, qs],
                                        avj[j][64 * j:64 * (j + 1), :], rec[:], Alu.mult)
            return finalize

        # One heavy PE op (AV matmul / QKV filler) is placed between
        # consecutive score groups so the exp stream never waits long for its
        # scores. The last three AV matmuls and the softmax-normalize of
        # iteration i spill into iteration i+1's early slots.
        pending = []   # deferred closures from the previous iteration
        it = 0
        for qc in range(QC):
            qs = slice(512 * qc, 512 * (qc + 1))
            if qc == 1:
                # qc0 projection drains through the filler slots of qc1
                def proj_filler(t):
                    return lambda: proj_unit(0, t)
                fillers += [proj_filler(t) for t in range(CT)]
            for p in range(CT):
                eb = expE[it % 2]
                kT, qT = qkT[4 + p], qkT[p]
                # avj[0]: rows 0:64 = head 2p AV, rows 64:128 = denom (replicated)
                # avj[1]: rows 0:64 = denom, rows 64:128 = head 2p+1 AV
                # 3-way av-bank rotation: reuse period 1.5 iterations, so the
                # finalize reads never block the next iteration's AV writes
                avj = [ps.tile([128, 512], f32, tag=f"av{j}",
                               name=f"av{j}") for j in range(2)]
                av_mm = make_av_mm(avj, eb, p)

                # Each slot emits its scores+exp FIRST (so the exp stream is
                # never queued behind heavy PE work), then one heavy PE op:
                # slots 0-2 drain the previous iteration's tail, 3-7 run this
                # iteration's AV as its exps land; QKV/proj fillers ride along.
                for kt in range(NT):
                    s01 = ps.tile([128, 2, 512], f32, tag=("sA" if kt % 2 == 0 else "sB"),
                                  name="s01")
                    for j in range(2):
                        nc.tensor.matmul(s01[:, j, :],
                                         kT[64 * j:64 * (j + 1), 128 * kt:128 * (kt + 1)],
                                         qT[64 * j:64 * (j + 1), qs],
                                         start=True, stop=True)
                    if kt in DVE_EXP_SLOTS:
                        # Schraudolph: fp8e4m3 bits of exp(s/8) ~ s*log2e + 56
                        # (constant rounding bias cancels in softmax)
                        nc.vector.tensor_scalar(eb[:, :, kt, :].bitcast(i8),
                                                s01[:], LOG2E, 56.0,
                                                Alu.mult, Alu.add)
                    else:
                        nc.scalar.activation(eb[:, :, kt, :], s01[:], Act.Exp,
                                             bias=0.0, scale=0.125)
                    if it == 0:
                        for f in it0_fillers.pop(kt, []):
                            f()
                    if kt <= 2 and pending:
                        pending.pop(0)()
                    if kt == 3:
                        av_mm(0, 0)
                    elif kt == 4:
                        av_mm(0, 1)
                    elif kt == 5:
                        av_mm(1, 0)
                    elif kt == 6:
                        av_mm(1, 1)
                    elif kt == 7:
                        av_mm(2, 0)
                    if it != 0 and kt in (2, 4, 6):
                        for _ in range(1 if kt == 2 else 2):
                            if fillers:
                                fillers.pop(0)()
                fin = make_finalize(avj, p, qs)
                pending = [lambda f=av_mm: f(2, 1),
                           lambda f=av_mm, g=fin: (f(3, 0), g(0)),
                           lambda f=av_mm, g=fin: (f(3, 1), g(1))]
                it += 1

        while pending:
            pending.pop(0)()
        for t in range(CT):
            proj_unit(1, t, tg=("mm", "sA", "sB", "mm")[t])

    nc.compile()
    return nc


def _get_nc():
    if "nc" not in _CACHE:
        _CACHE["nc"] = _build()
    return _CACHE["nc"]


def _prep_weights(gamma, beta, Wqkv, bqkv, Wproj, bproj):
    """Host-side constant prep shared across cores."""
    W = np.asarray(Wqkv, np.float32)
    bq = np.asarray(bqkv, np.float32) + np.asarray(beta, np.float32) @ W   # fold beta
    Wp = np.asarray(Wproj, np.float32)
    bv = bq[2 * C:3 * C]
    # residual-side constant: bproj + bv @ Wproj (added to masked rows on host)
    resid_bias = np.asarray(bproj, np.float32) + bv @ Wp

    # w8A: [128, 6144] = [wqk pair0 | wqk pair1 | wv pair0 | wv pair1]
    # wqk[pair][p, t, ot*128+m] = WS * W[256*pair + 128*t + p, qk-cout(ot, m)]
    w8A = np.zeros((128, 6144), np.float32)
    Wqk = W[:, 0:2 * C]        # q couts 0:512, k couts 512:1024
    Wv = W[:, 2 * C:3 * C]
    for pair in range(2):
        for t in range(2):
            rows = slice(256 * pair + 128 * t, 256 * pair + 128 * (t + 1))
            w8A[:, 2048 * pair + 1024 * t:2048 * pair + 1024 * (t + 1)] = Wqk[rows, :]
            w8A[:, 4096 + 1024 * pair + 512 * t:4096 + 1024 * pair + 512 * (t + 1)] = Wv[rows, :]
    w8A = (w8A * WS).astype(FP8)

    # w8B: [128, 2048] = [wp u0 | wp u1]; wp[u][p, t, c] = WS * Wproj[(2u+t)*128 + p, c]
    w8B = np.zeros((128, 2048), np.float32)
    for u in range(2):
        for t in range(2):
            rows = slice(128 * (2 * u + t), 128 * (2 * u + t + 1))
            w8B[:, 1024 * u + 512 * t:1024 * u + 512 * (t + 1)] = Wp[rows, :]
    w8B = (w8B * WS).astype(FP8)

    # cA: [128, 28 + 4*32] f32 = [bqk 0:8 | gam 8:12 | vmsk 12:20 | vraw 20:28 | sel]
    cA = np.zeros((128, 28 + CT * G), np.float32)
    for ot in range(8):
        cA[:, ot] = bq[128 * ot:128 * (ot + 1)]
    cA[:, 8:12] = np.asarray(gamma, np.float32).reshape(CT, 128).T
    for t in range(CT):
        for c in range(128):
            cA[c, 28 + G * t + (128 * t + c) // CPG] = 1.0

    sT = np.zeros((G, C + 1), np.float32)
    for c in range(C):
        sT[c // CPG, c] = 1.0
    return w8A, w8B, cA, sT, resid_bias


def kernel(x, lengths, gamma, beta, Wqkv, bqkv, Wproj, bproj):
    from concourse.bass_utils import run_bass_kernel_spmd

    x = np.asarray(x, np.float32)
    lengths = np.asarray(lengths).astype(np.int64)
    w8A, w8B, cA, sT, resid_bias = _prep_weights(gamma, beta, Wqkv, bqkv, Wproj, bproj)

    in_maps = []
    for s in range(B):
        ln = int(lengths[s])
        mf = (np.arange(L) < ln).astype(np.float32)
        xms = x[s] * mf[:, None]
        xmr = xms + mf[:, None] * resid_bias[None, :]
        # xmT [128, CT, L]: partition p, channel-tile t -> channel 128t+p
        xmT = np.ascontiguousarray(
            xms.T.reshape(CT, 128, L).transpose(1, 0, 2)).astype(BF16)
        # xm [128, NT, C]: partition p, token-tile t -> token 128t+p
        xmr8 = np.ascontiguousarray(
            xmr.reshape(NT, 128, C).transpose(1, 0, 2)).astype(BF16)
        cAs = cA.copy()
        cAs[:, 12:20] = mf.reshape(NT, 128).T / WS
        cAs[:, 20:28] = mf.reshape(NT, 128).T
        sTs = sT.copy()
        sTs[:, C] = 1.0 / max(ln * CPG, 1)
        qmv = np.where(mf > 0, np.float32(1.0), np.float32(1e30)).reshape(1, L)
        in_maps.append(dict(xmT=xmT, cA=cAs, sT=sTs, qmv=qmv, w8A=w8A,
                            w8B=w8B, xmr=xmr8))

    nc = _get_nc()
    res = run_bass_kernel_spmd(nc, in_maps, core_ids=list(range(B)))
    _CACHE["last_res"] = res
    # out [128, NT, C] -> [L, C]; padded rows zeroed on host
    out = np.stack([np.asarray(res.results[s]["out"]).transpose(1, 0, 2).reshape(L, C)
                    for s in range(B)], axis=0).astype(np.float32)
    for s in range(B):
        out[s, int(lengths[s]):] = 0.0
    return out


if __name__ == "__main__":
    rng = np.random.default_rng(0)
    x = rng.standard_normal((B, L, C), dtype=np.float32)
    lengths = rng.integers(L // 2, L + 1, size=(B,))
    gamma = np.ones(C, np.float32)
    beta = np.zeros(C, np.float32)
    Wqkv = (rng.standard_normal((C, 3 * C)) * 0.02).astype(np.float32)
    bqkv = np.zeros(3 * C, np.float32)
    Wproj = (rng.standard_normal((C, C)) * 0.02).astype(np.float32)
    bproj = np.zeros(C, np.float32)
    out = kernel(x=x, lengths=lengths, gamma=gamma, beta=beta, Wqkv=Wqkv,
                 bqkv=bqkv, Wproj=Wproj, bproj=bproj)
    print("out", out.shape, out.dtype, np.abs(out).max())
